# revision 17
# baseline (speedup 1.0000x reference)
"""GCN2 network (nn_GCNConvNet) on 8 Trainium2 NeuronCores via Bass/Tile.

Sharding: nodes are split into 8 contiguous shards (12500/core, padded to
12544 = 98*128).  Each core aggregates its own target nodes: incident edges
are fetched from a replicated feature table (rebuilt each layer with an
AllGather collective) using dma_gather, then reduced on the TensorEngine
with one-hot selector matmuls accumulated in PSUM (one [64,128] agg^T bank
per 128-node group).

Selector S[e, s] = (seg_e == s) * w_e is built on the VectorEngine from a
host-provided per-edge segment stream and weight stream, where
w_e = 0.9 * dinv[col_e] carries the GCN normalization of the target node
(dinv[row] is folded into the table rows: they store h' = dinv * h).
Padding edges get w_e = 0 so they contribute nothing.

The gather index operand is int16, so the 100352-row table is addressed in
4 chunks of 25088 rows; each core's edges are pre-sorted by (group, chunk)
and padded to 128-edge tiles with a core-uniform tile count (SPMD: one
program for all 8 cores).
"""
import sys

sys.path.insert(0, "/opt/trn_rl_repo")

import numpy as np

import concourse.bass as bass
import concourse.bacc as bacc
import concourse.mybir as mybir
import concourse.tile as tile
from concourse.bass_utils import run_bass_kernel_spmd

F32 = mybir.dt.float32
I16 = mybir.dt.int16
RELU = mybir.ActivationFunctionType.Relu

ALPHA = 0.1
NLAYERS = 4
NCHUNK = 4


class Cfg:
    """Host-derived compile-time constants of the SPMD program."""

    def __init__(self, n_nodes, pc, fin, fh, fout, batch, tgk, ni):
        self.n_nodes = n_nodes
        self.pc = pc
        self.n_cores = 8
        self.g = (pc + 127) // 128
        self.pcp = self.g * 128
        self.tab_rows = self.n_cores * self.pcp
        self.chunk = 2 * self.pcp
        assert self.chunk <= 32768
        assert NCHUNK * self.chunk == self.tab_rows
        self.fin = fin
        self.fh = fh
        self.fout = fout
        self.batch = batch              # idxs per gather instruction
        self.tpi = batch // 128         # tiles per instruction
        self.tgk = tgk                  # [g][k] tiles of group g from chunk k
        self.ni = ni                    # [k] gather instructions per stream
        self.sk = [ni[k] * self.tpi for k in range(NCHUNK)]   # stream tiles
        self.tile_base = np.concatenate([[0], np.cumsum(self.sk)]).astype(int)
        self.total_tiles = int(self.tile_base[-1])
        self.stream_w = sum(ni) * batch // 16
        self.xgrp = 8                   # fc1 x-slab size in groups


def build_nc(cfg: Cfg, debug_taps=False):
    nc = bacc.Bacc(
        "TRN2",
        target_bir_lowering=False,
        debug=False,
        num_devices=cfg.n_cores,
    )
    g, fh, fin, fout = cfg.g, cfg.fh, cfg.fin, cfg.fout
    pcp, tpi = cfg.pcp, cfg.tpi

    # ---- kernel I/O ----
    xT = nc.dram_tensor("xT", [fin, pcp], F32, kind="ExternalInput")
    w1T = nc.dram_tensor("w1T", [fin, fh], F32, kind="ExternalInput")
    b1b = nc.dram_tensor("b1b", [128, fh], F32, kind="ExternalInput")
    b1c = nc.dram_tensor("b1c", [fh, 1], F32, kind="ExternalInput")
    cw = nc.dram_tensor("cw", [fh, NLAYERS, fh], F32, kind="ExternalInput")
    w2T = nc.dram_tensor("w2T", [fh, fout], F32, kind="ExternalInput")
    b2b = nc.dram_tensor("b2b", [128, fout], F32, kind="ExternalInput")
    dinvi = nc.dram_tensor("dinv", [128, g], F32, kind="ExternalInput")
    iotai = nc.dram_tensor("iota", [128, 128], F32, kind="ExternalInput")
    gidx = nc.dram_tensor("gidx", [128, cfg.stream_w], I16, kind="ExternalInput")
    segi = nc.dram_tensor("seg", [128, cfg.total_tiles], F32,
                          kind="ExternalInput")
    w09i = nc.dram_tensor("w09", [128, cfg.total_tiles], F32,
                          kind="ExternalInput")
    outR = nc.dram_tensor("outR", [pcp, fout], F32, kind="ExternalOutput")
    if debug_taps:
        dbg_h0T = nc.dram_tensor("dbg_h0T", [fh, pcp], F32,
                                 kind="ExternalOutput")
        dbg_mt = nc.dram_tensor("dbg_mt", [fh, pcp], F32,
                                kind="ExternalOutput")
        dbg_tab = nc.dram_tensor("dbg_tab", [cfg.tab_rows, fh], F32,
                                 kind="ExternalOutput")

    with tile.TileContext(nc) as tc:
        with (
            tc.tile_pool(name="const", bufs=1) as cpool,
            tc.tile_pool(name="big", bufs=1) as bpool,
            tc.tile_pool(name="xin", bufs=2) as xpool,
            tc.tile_pool(name="gd0", bufs=2) as gp0,
            tc.tile_pool(name="gd1", bufs=2) as gp1,
            tc.tile_pool(name="gd2", bufs=2) as gp2,
            tc.tile_pool(name="gd3", bufs=2) as gp3,
            tc.tile_pool(name="sel", bufs=6) as selp,
            tc.tile_pool(name="mix", bufs=4) as mpool,
            tc.tile_pool(name="stage", bufs=4) as spool,
            tc.tile_pool(name="pagg", bufs=4, space="PSUM") as ps_agg,
            tc.tile_pool(name="prow", bufs=2, space="PSUM") as ps_row,
            tc.tile_pool(name="pout", bufs=2, space="PSUM") as ps_out,
            tc.tile_pool(name="dram", bufs=1, space="DRAM") as dpool,
        ):
            gpools = [gp0, gp1, gp2, gp3]
            # ---- internal DRAM ----
            cc_in = [
                dpool.tile([pcp, fh], F32, name=f"cc_in{i}") for i in range(NLAYERS)
            ]
            tables = [
                dpool.tile([cfg.tab_rows, fh], F32, addr_space="Shared",
                           name=f"table{i}")
                for i in range(NLAYERS)
            ]

            # ---- constants ----
            def cload(name, src, shape, dt=F32):
                t = cpool.tile(shape, dt, name=name)
                nc.sync.dma_start(t[:], src[:])
                return t

            w1T_s = cload("w1T_s", w1T, [fin, fh])
            b1b_s = cload("b1b_s", b1b, [128, fh])
            b1c_s = cload("b1c_s", b1c, [fh, 1])
            cw_s = cload("cw_s", cw, [fh, NLAYERS, fh])
            w2T_s = cload("w2T_s", w2T, [fh, fout])
            b2b_s = cload("b2b_s", b2b, [128, fout])
            dinv_s = cload("dinv_s", dinvi, [128, g])
            iota_s = cload("iota_s", iotai, [128, 128])
            gidx_s = cload("gidx_s", gidx, [128, cfg.stream_w], I16)
            seg_s = cload("seg_s", segi, [128, cfg.total_tiles])
            w09_s = cload("w09_s", w09i, [128, cfg.total_tiles])

            # h0T shared buffer: holds 0.1*h0 transposed; overwritten with
            # h4 transposed during layer 4.
            h0T = bpool.tile([fh, pcp], F32)

            # ---- fc1 ----
            nxch = (g + cfg.xgrp - 1) // cfg.xgrp
            for xc in range(nxch):
                g0 = xc * cfg.xgrp
                ng = min(cfg.xgrp, g - g0)
                xt = xpool.tile([fin, cfg.xgrp * 128], F32, tag="xt")
                nc.sync.dma_start(xt[:, : ng * 128],
                                  xT[:, g0 * 128:(g0 + ng) * 128])
                for j in range(ng):
                    gg = g0 + j
                    xsl = xt[:, j * 128:(j + 1) * 128]
                    # row-major: dinv*relu(x@W1+b1) -> cc_in[0]
                    pr = ps_row.tile([128, fh], F32, tag="pr")
                    nc.tensor.matmul(pr[:], xsl, w1T_s[:], start=True, stop=True)
                    nc.vector.tensor_add(pr[:], pr[:], b1b_s[:])
                    hp = spool.tile([128, fh], F32, tag="hp")
                    nc.scalar.activation(hp[:], pr[:], RELU,
                                         scale=dinv_s[:, gg:gg + 1])
                    nc.sync.dma_start(cc_in[0][gg * 128:(gg + 1) * 128, :], hp[:])
                    # transposed: 0.1*relu(x@W1+b1) -> h0T
                    pt = ps_agg.tile([fh, 128], F32, tag="pagg")
                    nc.tensor.matmul(pt[:], w1T_s[:], xsl, start=True, stop=True)
                    nc.scalar.activation(h0T[:, gg * 128:(gg + 1) * 128], pt[:],
                                         RELU, bias=b1c_s[:], scale=ALPHA)

            # ---- layers ----
            for l in range(1, NLAYERS + 1):
                tab = tables[l - 1]
                nc.gpsimd.collective_compute(
                    "AllGather",
                    mybir.AluOpType.bypass,
                    replica_groups=[list(range(cfg.n_cores))],
                    ins=[cc_in[l - 1].opt()],
                    outs=[tab.opt()],
                )

                # lazily-emitted gather instructions, per chunk stream
                emitted = [0, 0, 0, 0]
                gtiles = [None, None, None, None]

                def gather_upto(k, j, l=l, tab=tab, emitted=emitted,
                                gtiles=gtiles):
                    while emitted[k] <= j:
                        jj = emitted[k]
                        src = tab[k * cfg.chunk:(k + 1) * cfg.chunk, :]
                        off = (sum(cfg.ni[:k]) + jj) * (cfg.batch // 16)
                        gd = gpools[k].tile([128, tpi, fh], F32,
                                            tag=f"gd{k}", name=f"gd{k}_{l}_{jj}")
                        nc.gpsimd.dma_gather(
                            gd[:], src,
                            gidx_s[:, off:off + cfg.batch // 16],
                            cfg.batch, cfg.batch, fh,
                            single_packet=(cfg.batch // 16 + 1 <= 64),
                        )
                        if gtiles[k] is None or True:
                            pass
                        emitted[k] += 1
                        gtiles[k] = gtiles[k] or {}
                        gtiles[k][jj] = gd

                cursor = [0, 0, 0, 0]   # consumed tiles per stream
                for gg in range(g):
                    pagg = ps_agg.tile([fh, 128], F32, tag="pagg")
                    ntile_g = sum(cfg.tgk[gg])
                    done = 0
                    for k in range(NCHUNK):
                        for _ in range(cfg.tgk[gg][k]):
                            gt = cursor[k]
                            cursor[k] += 1
                            j, slot = gt // tpi, gt % tpi
                            gather_upto(k, j)
                            gd = gtiles[k][j]
                            col = int(cfg.tile_base[k]) + gt
                            sel = selp.tile([128, 128], F32, tag="sel")
                            nc.vector.tensor_scalar(
                                sel[:], iota_s[:],
                                seg_s[:, col:col + 1],
                                w09_s[:, col:col + 1],
                                op0=mybir.AluOpType.is_equal,
                                op1=mybir.AluOpType.mult)
                            nc.tensor.matmul(pagg[:], gd[:, slot, :], sel[:],
                                             start=(done == 0),
                                             stop=(done == ntile_g - 1))
                            done += 1
                    # mix: mixed^T = agg^T + 0.1*h0^T
                    mt = mpool.tile([fh, 128], F32, tag="mt")
                    nc.vector.tensor_add(mt[:], pagg[:],
                                         h0T[:, gg * 128:(gg + 1) * 128])
                    if debug_taps and l == 1:
                        nc.sync.dma_start(dbg_mt[:, gg * 128:(gg + 1) * 128],
                                          mt[:])
                        if gg == 0:
                            nc.sync.dma_start(dbg_tab[:], tab[:])
                            nc.sync.dma_start(dbg_h0T[:], h0T[:])
                    if l < NLAYERS:
                        pr = ps_row.tile([128, fh], F32, tag="pr")
                        nc.tensor.matmul(pr[:], mt[:], cw_s[:, l - 1, :],
                                         start=True, stop=True)
                        hp = spool.tile([128, fh], F32, tag="hp")
                        nc.scalar.activation(hp[:], pr[:], RELU,
                                             scale=dinv_s[:, gg:gg + 1])
                        nc.sync.dma_start(
                            cc_in[l][gg * 128:(gg + 1) * 128, :], hp[:])
                    else:
                        # h4^T = relu(W4^T @ mixed^T), overwrite h0T slice
                        pt4 = ps_agg.tile([fh, 128], F32, tag="pagg")
                        nc.tensor.matmul(pt4[:], cw_s[:, l - 1, :], mt[:],
                                         start=True, stop=True)
                        nc.scalar.activation(h0T[:, gg * 128:(gg + 1) * 128],
                                             pt4[:], RELU)

            # ---- fc2 ----
            for gg in range(g):
                po = ps_out.tile([128, fout], F32, tag="po")
                nc.tensor.matmul(po[:], h0T[:, gg * 128:(gg + 1) * 128],
                                 w2T_s[:], start=True, stop=True)
                ot = spool.tile([128, fout], F32, tag="ot")
                nc.vector.tensor_add(ot[:], po[:], b2b_s[:])
                nc.sync.dma_start(outR[gg * 128:(gg + 1) * 128, :], ot[:])

    nc.compile()
    return nc


def _wrap_idx(stream16):
    """[n] int16 -> [128, n//16]: edge i at (i%16, i//16), replicated 8x."""
    a = stream16.reshape(-1, 16).T.copy()
    return np.tile(a, (8, 1)).astype(np.int16)


def host_prep(x, edge_index, fc1_w, fc1_b, conv_w, fc2_w, fc2_b,
              n_nodes, pc, batch):
    fin = x.shape[1]
    fh = fc1_w.shape[0]
    fout = fc2_w.shape[0]
    n_cores = 8
    g = (pc + 127) // 128
    pcp = g * 128
    chunk = 2 * pcp
    tpi = batch // 128

    row = np.asarray(edge_index[0], dtype=np.int64)
    col = np.asarray(edge_index[1], dtype=np.int64)
    deg = np.bincount(col, minlength=n_nodes).astype(np.float32)
    with np.errstate(divide="ignore"):
        dinv = np.where(deg > 0, deg ** -0.5, 0.0).astype(np.float32)

    core_of = col // pc
    col_local = col - core_of * pc
    grp = col_local // 128
    seg_of = col_local - grp * 128
    tab_row = (row // pc) * pcp + (row % pc)
    ch_of = tab_row // chunk
    rel = (tab_row - ch_of * chunk).astype(np.int16)

    # per (core, group, chunk) edge counts
    key = (core_of * g + grp) * NCHUNK + ch_of
    counts = np.bincount(key, minlength=n_cores * g * NCHUNK)
    counts = counts.reshape(n_cores, g, NCHUNK)
    tgk = np.ceil(counts.max(axis=0) / 128).astype(np.int64)   # [g, NCHUNK]
    tgk[0, 0] = max(tgk[0, 0], 1)
    sk_tiles = tgk.sum(axis=0)                                 # [NCHUNK]
    ni = [int(np.ceil(sk_tiles[k] * 128 / batch)) for k in range(NCHUNK)]

    cfg = Cfg(n_nodes, pc, fin, fh, fout, batch,
              tgk.tolist(), ni)

    # tile offset of (g, k) within stream k
    toff = np.zeros((g, NCHUNK), dtype=np.int64)
    toff[1:] = np.cumsum(tgk, axis=0)[:-1]

    order_all = np.lexsort((col_local, ch_of, grp, core_of))
    bounds = np.searchsorted(core_of[order_all], np.arange(n_cores + 1))

    w1Tn = np.ascontiguousarray(fc1_w.T).astype(np.float32)
    b1bn = np.tile(fc1_b.reshape(1, fh), (128, 1)).astype(np.float32)
    # fc1 transposed pass computes relu(ALPHA*(x@W1.T) + bias), so the bias
    # must be pre-scaled: ALPHA*relu(y) == relu(ALPHA*y) for ALPHA > 0.
    b1cn = (ALPHA * fc1_b).reshape(fh, 1).astype(np.float32)
    cwn = np.ascontiguousarray(conv_w.transpose(1, 0, 2)).astype(np.float32)
    w2Tn = np.ascontiguousarray(fc2_w.T).astype(np.float32)
    b2bn = np.tile(fc2_b.reshape(1, fout), (128, 1)).astype(np.float32)
    iota = np.tile(np.arange(128, dtype=np.float32).reshape(1, 128), (128, 1))

    in_maps = []
    for c in range(n_cores):
        oc = order_all[bounds[c]:bounds[c + 1]]
        # streams per chunk, with per-(g,k) padding to the uniform tile count
        gstreams = []
        segv = np.zeros((cfg.total_tiles, 128), dtype=np.float32)
        w09v = np.zeros((cfg.total_tiles, 128), dtype=np.float32)
        okey = grp[oc] * NCHUNK + ch_of[oc]
        ob = np.searchsorted(okey, np.arange(g * NCHUNK + 1))
        for k in range(NCHUNK):
            st = np.zeros(cfg.sk[k] * 128, dtype=np.int16)
            st[:] = ((np.arange(cfg.sk[k] * 128) * 61) % chunk).astype(np.int16)
            for gg in range(g):
                ek = oc[ob[gg * NCHUNK + k]:ob[gg * NCHUNK + k + 1]]
                ne = len(ek)
                base = toff[gg, k] * 128
                st[base:base + ne] = rel[ek]
                tb = int(cfg.tile_base[k]) + toff[gg, k]
                nt = tgk[gg, k]
                sflat = np.zeros(nt * 128, dtype=np.float32)
                wflat = np.zeros(nt * 128, dtype=np.float32)
                sflat[:ne] = seg_of[ek]
                wflat[:ne] = (1.0 - ALPHA) * dinv[col[ek]]
                segv[tb:tb + nt] = sflat.reshape(nt, 128)
                w09v[tb:tb + nt] = wflat.reshape(nt, 128)
            gstreams.append(st)
        gstream = np.concatenate(gstreams)

        n0 = c * pc
        xs = np.zeros((fin, pcp), dtype=np.float32)
        xs[:, :pc] = np.ascontiguousarray(x[n0:n0 + pc].T)
        dinvc = np.zeros(pcp, dtype=np.float32)
        dinvc[:pc] = dinv[n0:n0 + pc]
        in_maps.append({
            "xT": xs,
            "w1T": w1Tn,
            "b1b": b1bn,
            "b1c": b1cn,
            "cw": cwn,
            "w2T": w2Tn,
            "b2b": b2bn,
            "dinv": np.ascontiguousarray(
                dinvc.reshape(g, 128).T).astype(np.float32),
            "iota": iota,
            "gidx": _wrap_idx(gstream),
            "seg": np.ascontiguousarray(segv.T),
            "w09": np.ascontiguousarray(w09v.T),
        })
    return cfg, in_maps


_CACHE = {}


def _get_nc(key, cfg):
    if key not in _CACHE:
        _CACHE[key] = build_nc(cfg)
    return _CACHE[key]


def kernel(x, edge_index, batch_graph, fc1_w, fc1_b, conv_w, fc2_w, fc2_b):
    x = np.asarray(x, dtype=np.float32)
    n_nodes = x.shape[0]
    pc = n_nodes // 8
    cfg, in_maps = host_prep(
        x, edge_index, np.asarray(fc1_w), np.asarray(fc1_b),
        np.asarray(conv_w), np.asarray(fc2_w), np.asarray(fc2_b),
        n_nodes, pc, batch=2048)
    nc = _get_nc(("full", n_nodes, tuple(cfg.ni),
                  tuple(map(tuple, cfg.tgk))), cfg)
    res = run_bass_kernel_spmd(nc, in_maps, core_ids=list(range(8)))
    outs = []
    for c in range(8):
        outs.append(res.results[c]["outR"][:pc, :])
    return np.ascontiguousarray(np.concatenate(outs, axis=0)).astype(np.float32)
